# revision 36
# baseline (speedup 1.0000x reference)
"""Trainium2 kernel for nn_MultiHeadCrossAttention_28063316313030.

Math: with seq_len == 1, softmax over a size-1 axis is identically 1, so
attention(Q,K,V) == V and W_Q/W_K are dead code.  The whole module collapses to

    out = LN(x1 @ A) + LN(x2 @ A),   A = W_V.T @ W_fc.T   (1024 x 1024)

where LN is LayerNorm over the last dim with gamma/beta.

Distribution: pure data parallel over the batch dim across 8 NeuronCores.
Host precomputes A (tiny matmul) and pre-tiles x1/x2 C-major so the TensorE
contraction dim lands on SBUF partitions with fully contiguous DMA runs.

Device per core (2048 rows per stream), bf16 matmuls (rel err ~2.6e-3 vs the
2e-2 gate), f32 PSUM accumulation.  Design notes from trace analysis:
  * bf16 vs f32r matmul: same 1 cycle/row PE rate, but bf16 LDWEIGHTS
    (~95ns) hides fully under 512-row matmuls (~216ns cadence, peak), while
    f32r LDWEIGHTS (~190ns) leaks ~27ns/matmul.  bf16 also halves x/A DMA.
  * The PE p-state ramp (0.65 -> 1.2 -> 2.4GHz after ~3us of CONTINUOUS
    busy) resets on any idle gap.  N_WARMUP dummy matmuls on a memset tile
    (no DMA dependency) bridge from the engine preamble to the moment real
    data lands, so real matmuls start at full clock and never gap.
  * The first ~12us are HBM-bandwidth-critical: A (2MB) must stream at
    ~290GB/s to keep bt0's k-major consumption fed.  x DMA issues go on
    GpSimd (A owns Sync); GpSimd memsets of a scratch tile delay bt1's
    singles and pair 1 so eager x traffic cannot starve the A stream; pairs
    2..7 are paced by sitting behind each tile's h1 out-DMA in GpSimd
    program order.  Each DMA descriptor write costs ~0.6us on its engine.
  * Tiles are s-BLOCKED (all of stream 0's matmuls, then stream 1's) so
    stream 0's LayerNorm chain (bn_stats/bn_aggr, sqrt+recip, ACT
    normalize) overlaps stream 1's matmuls and PSUM banks release well
    inside the 2-buffer slack.  Final add on DVE; output via Sync/GpSimd.
  * Tail: the last tile's h1 add runs as two 256-wide chunks, each DMA'd
    immediately from a different queue (Scalar/Sync), shortening the final
    drain.  ~430ns PE stalls every ~10.8us are HBM refresh (unavoidable).
  * fp8 fails accuracy: measured 6.1e-2 rel err full-fp8; even 1/8 of K in
    fp8 scales to ~2.2e-2 > 2e-2 gate.
"""

import sys

sys.path.insert(0, "/opt/trn_rl_repo")

import numpy as np

B, C, OUT = 16384, 1024, 1024
EPS = 1e-5
NCORES = 8
R = B // NCORES  # rows per core per stream
P = 128
KT = C // P  # contraction tiles
BT = R // P  # row tiles per core
NH = OUT // 512  # psum bank halves per row tile
N_WARMUP = 14

_cache = {}


def _build(use_affine: bool, mm_dtype_name: str):
    import concourse.bacc as bacc
    import concourse.bass as bass
    import concourse.mybir as mybir
    from concourse.tile import TileContext

    f32 = mybir.dt.float32
    mmdt = getattr(mybir.dt, mm_dtype_name)
    AF = mybir.ActivationFunctionType
    ALU = mybir.AluOpType

    nc = bacc.Bacc("TRN2", target_bir_lowering=False, debug=False, num_devices=NCORES)

    # host-pretiled: [ki, bt, ko, bi]
    x1p = nc.declare_dram_parameter("x1p", [P, BT, KT, P], mmdt, isOutput=False)
    x2p = nc.declare_dram_parameter("x2p", [P, BT, KT, P], mmdt, isOutput=False)
    # host-pretiled: [ki, ko, o]
    a_d = nc.declare_dram_parameter("a", [P, KT, OUT], mmdt, isOutput=False)
    if use_affine:
        gam_d = nc.declare_dram_parameter("gamma", [OUT], f32, isOutput=False)
        bet2_d = nc.declare_dram_parameter("beta2", [OUT], f32, isOutput=False)
    y_d = nc.declare_dram_parameter("y", [R, OUT], f32, isOutput=True)

    with TileContext(nc) as tc:
        with (
            tc.tile_pool(name="singles", bufs=1) as singles,
            tc.tile_pool(name="xs", bufs=2) as xpool,
            tc.tile_pool(name="ns", bufs=3) as npool,
            tc.tile_pool(name="outs", bufs=3) as opool,
            tc.tile_pool(name="stats", bufs=4) as stats,
            tc.tile_pool(name="psum", bufs=2, space="PSUM") as psum,
        ):
            # --- PE warmup on a memset tile: zero DMA dependency, so the
            # p-state ramp (LOW->MID->full after ~3us of continuous PE
            # activity) completes while the first real tiles stream in.
            # Vector's preamble ends earliest and GpSimd must stay free to
            # fire the x-tile DMA descriptors.
            warm_sb = singles.tile([P, 512], mmdt)
            nc.vector.memset(warm_sb, 0.25)
            warm_ps = psum.tile([P, 512], f32, tag="ps11")
            for w in range(N_WARMUP):
                lo = 128 * (w % 2)
                nc.tensor.matmul(
                    warm_ps[:], lhsT=warm_sb[:, lo : lo + P], rhs=warm_sb[:],
                    start=True, stop=True,
                )

            # --- x tiles: bt0/bt1 as single tiles (smallest possible first
            # dependency), then PAIR tiles.  All issued on GpSimd so the Sync
            # queue is dedicated to the A stream at startup.
            # GpSimd memsets of a scratch tile act as time-delays in the x
            # issue queue: bt1's singles and pair 1 reach the DMA engines
            # only once the A stream has had the HBM mostly to itself.  (The
            # scheduler may interleave them, but pair 1 stays last.)
            xdelay = singles.tile([P, 2048], f32, name="xdelay")
            xt_single = {}
            for bt in range(2):
                for s, xp in enumerate((x1p, x2p)):
                    t = singles.tile([P, KT, P], mmdt, name=f"xts{bt}_{s}")
                    nc.gpsimd.dma_start(t[:], xp[:, bt])
                    xt_single[(bt, s)] = t
                if bt == 0:
                    for _ in range(4):
                        nc.gpsimd.memset(xdelay, 0.0)

            # --- A: five contiguous DMAs on the dedicated Sync queue, sized
            # so FIFO delivery tracks bt0's k-major consumption: k=0 in
            # halves (the very first matmul waits on only 128KB), then
            # k-groups.  (h-split strided A transfers measured ~2x slower --
            # keep these contiguous.)
            a0h = []
            for h in range(NH):
                t = singles.tile([P, 512], mmdt, name=f"a0_{h}")
                nc.sync.dma_start(t[:], a_d[:, 0, h * 512 : (h + 1) * 512])
                a0h.append(t)
            a_groups = [(1, 3), (3, 6), (6, 8)]
            a_gt = []
            for g, (k0, k1) in enumerate(a_groups):
                t = singles.tile([P, k1 - k0, OUT], mmdt, name=f"ag{g}")
                nc.sync.dma_start(t[:], a_d[:, k0:k1])
                a_gt.append(t)

            def rhs_a(k, h):
                if k == 0:
                    return a0h[h][:]
                for g, (k0, k1) in enumerate(a_groups):
                    if k0 <= k < k1:
                        return a_gt[g][:, k - k0, h * 512 : (h + 1) * 512]

            eps_sb = singles.tile([P, 1], f32)
            nc.vector.memset(eps_sb, EPS)
            if use_affine:
                gam_sb = singles.tile([P, OUT], f32)
                nc.sync.dma_start(
                    gam_sb[:],
                    bass.AP(
                        tensor=gam_d.tensor,
                        offset=gam_d.offset,
                        ap=[[0, P], gam_d.ap[0]],
                    ),
                )
                bet2_sb = singles.tile([P, OUT], f32)
                nc.sync.dma_start(
                    bet2_sb[:],
                    bass.AP(
                        tensor=bet2_d.tensor,
                        offset=bet2_d.offset,
                        ap=[[0, P], bet2_d.ap[0]],
                    ),
                )

            # remaining x tiles as PAIRS (one 512KB DMA per 2 row tiles).
            # Only pair 1 is eager; pairs 2..7 are issued from inside the
            # tile loop (paced by a tiny GpSimd op that depends on an earlier
            # tile's stats) so eager x traffic cannot starve the A stream of
            # HBM bandwidth during the clock-ramp-critical first 20us.
            xt_pair = {}

            def issue_pair(j):
                for s, xp in enumerate((x1p, x2p)):
                    t = xpool.tile(
                        [P, 2, KT, P], mmdt, tag=f"xt{s}", name=f"xtp{j}_{s}"
                    )
                    nc.gpsimd.dma_start(t[:], xp[:, 2 * j : 2 * j + 2])
                    xt_pair[(j, s)] = t

            for _ in range(3):
                nc.gpsimd.memset(xdelay, 0.0)
            issue_pair(1)

            def lhs_x(bt, s):
                if bt < 2:
                    return xt_single[(bt, s)][:, :, :]
                return xt_pair[(bt // 2, s)][:, bt % 2]

            def stream_stats(bt, s, ps_tiles):
                """bn stats -> r = 1/sqrt(var+eps), nmr = -mu*r for stream s."""
                st = stats.tile([P, NH, 6], f32, tag=f"st{s}", name=f"st{bt}{s}")
                mv = stats.tile([P, 2], f32, tag=f"mv{s}", name=f"mv{bt}{s}")
                r_sb = stats.tile([P, 1], f32, tag=f"r{s}", name=f"r{bt}{s}")
                nmr = stats.tile([P, 1], f32, tag=f"nmr{s}", name=f"nmr{bt}{s}")
                for h in range(NH):
                    nc.vector.bn_stats(st[:, h, :], ps_tiles[h][:])
                nc.vector.bn_aggr(mv[:], st[:])
                nc.scalar.activation(
                    r_sb[:], mv[:, 1:2], func=AF.Sqrt, bias=eps_sb[:], scale=1.0
                )
                nc.vector.reciprocal(r_sb[:], r_sb[:])
                nc.vector.tensor_scalar(
                    nmr[:],
                    mv[:, 0:1],
                    scalar1=r_sb[:],
                    scalar2=-1.0,
                    op0=ALU.mult,
                    op1=ALU.mult,
                )
                return r_sb, nmr

            def normalize(bt, s, ps_tiles, r_sb, nmr):
                """n_s = z_s * r_s - mu_s*r_s via ACT (per-partition scale/bias)."""
                n = npool.tile([P, OUT], f32, tag=f"n{s}", name=f"n{bt}_{s}")
                for h in range(NH):
                    nc.scalar.activation(
                        n[:, h * 512 : (h + 1) * 512], ps_tiles[h][:],
                        func=AF.Identity, bias=nmr[:], scale=r_sb[:],
                    )
                return n

            def mm_block(bt, s, ps_s, k_order=None):
                for k in range(KT):
                    for h in range(NH):
                        nc.tensor.matmul(
                            ps_s[h][:],
                            lhsT=lhs_x(bt, s)[:, k, :],
                            rhs=rhs_a(k, h),
                            start=(k == 0),
                            stop=(k == KT - 1),
                        )

            for bt in range(BT):
                last = bt == BT - 1
                ps = {
                    s: [
                        psum.tile([P, 512], f32, tag=f"ps{s}{h}", name=f"ps{bt}{s}{h}")
                        for h in range(NH)
                    ]
                    for s in range(2)
                }

                if bt == 0:
                    # k-major across both streams: 4 matmuls per A chunk, so
                    # the PE keeps pace with the A DMA stream at kernel start.
                    for k in range(KT):
                        for s in range(2):
                            for h in range(NH):
                                nc.tensor.matmul(
                                    ps[s][h][:],
                                    lhsT=lhs_x(bt, s)[:, k, :],
                                    rhs=rhs_a(k, h),
                                    start=(k == 0),
                                    stop=(k == KT - 1),
                                )
                    r0, nmr0 = stream_stats(bt, 0, ps[0])
                    n1 = normalize(bt, 0, ps[0], r0, nmr0)
                    r1, nmr1 = stream_stats(bt, 1, ps[1])
                    n2 = normalize(bt, 1, ps[1], r1, nmr1)
                else:
                    # s-blocked: stream 0's epilogue overlaps stream 1's
                    # matmuls, halving PSUM release latency.  (A tail
                    # variant with stream 1's h1 as two 256-wide groups in
                    # separate psum banks shortened the final stats chain
                    # but lost more to the extra ACT pair -- reverted.)
                    mm_block(bt, 0, ps[0])
                    r0, nmr0 = stream_stats(bt, 0, ps[0])
                    n1 = normalize(bt, 0, ps[0], r0, nmr0)
                    mm_block(bt, 1, ps[1])
                    r1, nmr1 = stream_stats(bt, 1, ps[1])
                    n2 = normalize(bt, 1, ps[1], r1, nmr1)

                out_t = opool.tile([P, OUT], f32, tag="out", name=f"out{bt}")
                rows = slice(bt * P, (bt + 1) * P)
                # h0 leaves via Sync (idle once A has streamed in), h1 via
                # GpSimd.  For the very last tile, h1's add+DMA runs as two
                # 256-wide chunks on two queues so the final transfer starts
                # as early as possible.
                h_chunks = [(slice(0, 512), nc.sync)]
                if last:
                    h_chunks += [
                        (slice(512, 768), nc.scalar),
                        (slice(768, 1024), nc.sync),
                    ]
                else:
                    h_chunks += [(slice(512, 1024), nc.gpsimd)]
                for sl, eng_dma in h_chunks:
                    nc.vector.tensor_tensor(
                        out_t[:, sl], n1[:, sl], n2[:, sl], op=ALU.add
                    )
                    if use_affine:
                        nc.vector.tensor_tensor(
                            out_t[:, sl], out_t[:, sl], gam_sb[:, sl], op=ALU.mult
                        )
                        nc.vector.tensor_tensor(
                            out_t[:, sl], out_t[:, sl], bet2_sb[:, sl], op=ALU.add
                        )
                    eng_dma.dma_start(y_d[rows, sl], out_t[:, sl])

                # paced x-pair DMA issues: pair j=bt//2+2 is emitted after
                # this tile's h1 out-DMA on the GpSimd queue, whose data
                # dependency (this tile's add) delays it -- eager x traffic
                # would otherwise starve the A stream of early HBM bandwidth
                # while staying ~4 tiles ahead of use.
                if bt % 2 == 0 and 2 <= bt // 2 + 2 < BT // 2:
                    issue_pair(bt // 2 + 2)

    nc.finalize()
    return nc


def _get_nc(use_affine: bool, mm_dtype_name: str):
    key = (use_affine, mm_dtype_name)
    if key not in _cache:
        _cache[key] = _build(use_affine, mm_dtype_name)
    return _cache[key]


def _pretile_x(x_core: np.ndarray, np_mm) -> np.ndarray:
    # [R, C] -> [ki, bt, ko, bi]
    return np.ascontiguousarray(
        x_core.reshape(BT, P, KT, P).transpose(3, 0, 2, 1).astype(np_mm)
    )


def kernel(x1, x2, W_Q, W_K, W_V, W_fc, gamma, beta, _trace=False,
           _mm_dtype="bfloat16"):
    from concourse.bass_utils import run_bass_kernel_spmd

    x1 = np.asarray(x1, dtype=np.float32)
    x2 = np.asarray(x2, dtype=np.float32)
    W_V = np.asarray(W_V, dtype=np.float32)
    W_fc = np.asarray(W_fc, dtype=np.float32)
    gamma = np.asarray(gamma, dtype=np.float32)
    beta = np.asarray(beta, dtype=np.float32)

    # A = W_V.T @ W_fc.T in float64 to keep the host collapse error negligible.
    A = (W_V.T.astype(np.float64) @ W_fc.T.astype(np.float64)).astype(np.float32)
    # [C, OUT] -> [ki, ko, o]
    Ap = np.ascontiguousarray(A.reshape(KT, P, OUT).transpose(1, 0, 2))

    use_affine = not (np.all(gamma == 1.0) and np.all(beta == 0.0))

    if _mm_dtype == "bfloat16":
        import ml_dtypes

        np_mm = ml_dtypes.bfloat16
    else:
        np_mm = np.float32
    Ap = Ap.astype(np_mm)

    in_maps = []
    for r in range(NCORES):
        sl = slice(r * R, (r + 1) * R)
        m = {
            "x1p": _pretile_x(x1[sl], np_mm),
            "x2p": _pretile_x(x2[sl], np_mm),
            "a": Ap,
        }
        if use_affine:
            m["gamma"] = gamma
            m["beta2"] = (2.0 * beta).astype(np.float32)
        in_maps.append(m)

    nc = _get_nc(use_affine, _mm_dtype)
    res = run_bass_kernel_spmd(nc, in_maps, list(range(NCORES)), trace=_trace)

    y = np.concatenate([res.results[r]["y"] for r in range(NCORES)], axis=0)
    out = y.reshape(B, 1, OUT)
    if _trace:
        return out, res
    return out
